# revision 9
# baseline (speedup 1.0000x reference)
"""Group-wise correlation cost volume (build_gwc_volume) on 8 trn2 cores.

volume[b,g,d,h,w] = sum_c ref[b,g,c,h,w] * tgt[b,g,c,h,w-d]  (0 where w<d)

Sharding: 16 (b,g) pairs across 8 cores, 2 pairs per core. Each pair is a
contiguous 64-channel slice of the inputs and a contiguous [D,H,W] slab of
the output.

Per (b,g,h) the volume rows are diagonals of the Gram matrix
G[w',w] = sum_c tgt[c,w'] * ref[c,w].  Only the band d = w - w' in [0,48)
is needed, so the Gram is computed as 8 column-piece matmuls (M=32,
stationary T[:, 32k:32k+32]) whose moving window is R[:, 32k : 32k+WID_k),
WID_k = min(79, W - 32k) — the minimal window covering the band for the
piece (clipped at the right image edge).  Piece k lands at PSUM rows
32*(k%4), col half 0/79; the two (b,g) pairs sit on PE row halves so all
piece-position slots of the 128x128 array are used.

All HBM traffic is fp16.  ref/tgt are interleaved host-side into one
[pr, c, 2, h, w] tensor so each h-chunk needs a single input DMA, and both
pairs' band tiles share one [row, pr, h, x] output tensor so each chunk
needs a single output DMA — fewer DMAs means the Tile scheduler's 8 DMA
semaphore lanes recycle with a longer reuse distance, which keeps input
prefetch from serializing behind output completions.

Diagonal (shear) extraction at 1-partition granularity is not expressible
in any engine's access patterns, so the band tiles are DMAed out and the
diagonals are gathered on the host during unsharding.
"""

import sys

if "/opt/trn_rl_repo" not in sys.path:
    sys.path.insert(0, "/opt/trn_rl_repo")

import numpy as np

import concourse.bacc as bacc
import concourse.tile as tile
from concourse import mybir
from concourse.bass_utils import run_bass_kernel_spmd

F32 = mybir.dt.float32
F16 = mybir.dt.float16

B, C, H, W = 2, 512, 128, 256
G, CG, D = 8, 64, 48
N_CORES = 8
PAIRS = 2  # (b,g) pairs per core
HC = 8  # h rows per chunk
PW = 79  # full piece window width (32 + 47)
BW = 2 * PW  # band tile width (two col halves)

# piece k covers w' in [32k, 32k+32); its moving window is
# [32k, 32k + WID[k]) — the band needs w in [w', w'+47], clipped at W.
WID = [min(PW, W - 32 * k) for k in range(8)]

_cached = {}


def _build_module():
    nc = bacc.Bacc("TRN2", target_bir_lowering=False, debug=False, num_devices=N_CORES)
    # ref/tgt interleaved on axis 2: [..., 0, ...] = ref, [..., 1, ...] = tgt
    rt_in = nc.dram_tensor("rt_in", [PAIRS, CG, 2, H, W], F16, kind="ExternalInput")
    # band tiles, layout [w'-row, pair, h, x]: cols 0:79 pieces 0-3
    # (w' in [0,128)), cols 79:158 pieces 4-7 (w' in [128,256))
    out_bt = nc.dram_tensor(
        "out_bt", [128, PAIRS, H, BW], F16, kind="ExternalOutput"
    )

    rt_p = rt_in.rearrange("pr c two h w -> (pr c) two h w")

    with tile.TileContext(nc) as tc:
        with (
            tc.tile_pool(name="ins", bufs=10) as ins,
            tc.tile_pool(name="stage", bufs=5) as stage_pool,
            tc.tile_pool(name="psum", bufs=4, space="PSUM") as psum,
        ):
            for ch in range(H // HC):
                h0 = ch * HC
                it = ins.tile([128, 2, HC, W], F16, tag="it")
                nc.sync.dma_start(it[:], rt_p[:, :, h0 : h0 + HC, :])
                st = stage_pool.tile(
                    [128, PAIRS, HC, BW], F16, tag="st", name=f"st_{ch}"
                )
                for hg in range(HC // 2):  # two h rows per PSUM bank
                    for pr in range(PAIRS):
                        p0 = pr * CG
                        bank = psum.tile(
                            [128, 2, BW], F32, tag=f"bk{pr}", name=f"bk{pr}_{ch}_{hg}"
                        )
                        for j in range(2):
                            hl = 2 * hg + j
                            for k in range(8):
                                c0 = PW * (k // 4)
                                m0 = 32 * (k % 4)
                                w0 = 32 * k
                                nc.tensor.matmul(
                                    bank[m0 : m0 + 32, j, c0 : c0 + WID[k]],
                                    it[p0 : p0 + CG, 1, hl, w0 : w0 + 32],
                                    it[p0 : p0 + CG, 0, hl, w0 : w0 + WID[k]],
                                    tile_position=(p0, m0),
                                )
                        eng = nc.vector if (hg + pr) % 2 == 0 else nc.scalar
                        copy = eng.tensor_copy if eng is nc.vector else eng.copy
                        copy(st[:, pr, 2 * hg : 2 * hg + 2, :], bank[:, :, :])
                nc.scalar.dma_start(out_bt[:, :, h0 : h0 + HC, :], st[:])

    nc.compile()
    return nc


def _get_module():
    if "nc" not in _cached:
        _cached["nc"] = _build_module()
    return _cached["nc"]


def _make_in_maps(refimg_fea, targetimg_fea):
    ref = np.ascontiguousarray(refimg_fea, dtype=np.float32).astype(np.float16)
    tgt = np.ascontiguousarray(targetimg_fea, dtype=np.float32).astype(np.float16)
    rp = ref.reshape(B * G, CG, H, W)
    tp = tgt.reshape(B * G, CG, H, W)
    rt = np.ascontiguousarray(
        np.stack([rp, tp], axis=2)
    )  # [16, CG, 2, H, W]
    return [{"rt_in": rt[2 * k : 2 * k + 2]} for k in range(N_CORES)]


def _host_extract(bt):
    """Gather band diagonals into the full volume.

    bt: [16, 128, H, 158] (pair-major).  Piece k = w'//32 puts G[w', w] at
    row w' % 128, col 79*(k//4) + (w - 32k), valid for w - 32k in
    [0, WID[k]).  vol[d,h,w] = G[w-d, w].
    """
    d = np.arange(D)[:, None]
    w = np.arange(W)[None, :]
    wp = w - d  # [D, W] source w' (negative -> zero region)
    valid = wp >= 0
    wpc = np.clip(wp, 0, None)
    k = wpc // 32
    col = PW * (k // 4) + (w - 32 * k)
    row = wpc % 128
    assert (col[valid] >= PW * (k[valid] // 4)).all() and (
        col[valid] < PW * (k[valid] // 4) + np.asarray(WID)[k[valid]]
    ).all()

    vol = np.zeros((B * G, D, H, W), np.float32)
    for pair in range(B * G):
        t = bt[pair].transpose(1, 0, 2)  # [h, row, col]
        r = t[:, row, col].astype(np.float32)  # [H, D, W]
        r *= valid[None]
        vol[pair] = r.transpose(1, 0, 2)
    return vol.reshape(B, G, D, H, W)


def kernel(refimg_fea, targetimg_fea, num_groups, maxdisp):
    assert int(num_groups) == G and int(maxdisp) == D
    assert tuple(refimg_fea.shape) == (B, C, H, W)

    in_maps = _make_in_maps(refimg_fea, targetimg_fea)

    nc = _get_module()
    res = run_bass_kernel_spmd(nc, in_maps, core_ids=list(range(N_CORES)))

    # out_bt per core: [128, PAIRS, H, BW] -> pair-major [2*PAIRS.., 128, H, BW]
    bt = np.concatenate(
        [r["out_bt"].transpose(1, 0, 2, 3) for r in res.results], axis=0
    )
    return _host_extract(bt)
